# revision 15
# baseline (speedup 1.0000x reference)
"""BiMamba (bidirectional Mamba v1) TRN2 Bass kernel, 8-core SPMD.

Shapes (hardcoded): B=1, L=2048, Dm=1024, Di=2048, N=16, K=4, R=64.
Sharding: tensor-parallel over d_inner across 8 cores (Di_loc=256).
Collectives: AllReduce of x_dbl (dt/B/C projections), ReduceScatter of out^T
(both in bf16).

Scan layout: per-(dir, state) tiles [128 channels, L].  dA comes straight
from Exp(dt * A[:, n]) with a per-partition scale AP; B/C rows are broadcast
from the AllReduce result in DRAM to all 128 partitions by cast-DMAs; the
n-contraction is an identity matmul accumulating into a PSUM py tile.  The
backward direction is the same scan with negative strides.
"""
import sys

sys.path.insert(0, '/opt/trn_rl_repo')

import numpy as np

import concourse.bacc as bacc
import concourse.mybir as mybir
from concourse.bass_utils import run_bass_kernel_spmd
from concourse.tile import TileContext

AF = mybir.ActivationFunctionType
ALU = mybir.AluOpType
F32 = mybir.dt.float32
BF16 = mybir.dt.bfloat16

L = 2048
DM = 1024
DI = 2048
N = 16
KC = 4
R = 64
NCORES = 8
DLOC = DI // NCORES      # 256
NG = DLOC // 128         # 2
CH = 512
TC4 = L // CH            # 4
PADL = KC - 1            # 3
XPW = L + 2 * PADL

_CACHE = {}


def _emit(nc, t):
    with TileContext(nc) as tc:
        with tc.tile_pool(name='sb', bufs=1) as P, \
             tc.tile_pool(name='ps', bufs=1, space='PSUM') as PS:

            # ---- small persistent loads ----
            def load2(name, src, cols, dt=F32):
                out = []
                for g in range(NG):
                    tl = P.tile([128, cols], dt, tag=name, bufs=2,
                                name=f'{name}{g}')
                    nc.sync.dma_start(out=tl[:],
                                      in_=src[g * 128:(g + 1) * 128, :])
                    out.append(tl)
                return out

            bdt_g = load2('bdt', t['bdt'], 2)            # fwd | rev
            convw_g = load2('convw', t['conv_w'], 2 * KC)
            convb_g = load2('convb', t['conv_b'], 2)
            d_g = load2('dboth', t['d_both'], 2)
            asc_g = load2('asc', t['a_sc'], 2 * N)       # [128, 32] f32
            ident = P.tile([128, 128], BF16, tag='ident', bufs=1, name='ident')
            nc.sync.dma_start(out=ident[:], in_=t['ident'][:])
            wx_g = []
            for g in range(NG):
                wxt = P.tile([128, 2 * 96], BF16, tag='wx', bufs=2,
                             name=f'wx{g}')
                nc.sync.dma_start(out=wxt[:],
                                  in_=t['wx_T'][g * 128:(g + 1) * 128, :])
                wx_g.append(wxt)
            wdt_t = P.tile([R, 2 * DLOC], BF16, tag='wdt', bufs=1, name='wdt_t')
            nc.sync.dma_start(out=wdt_t[:], in_=t['wdt_T'][:])

            # ---- in_proj ----
            x_pad = []
            z_t = []
            for g in range(NG):
                xp = P.tile([128, XPW], BF16, tag='xpad', bufs=2, name=f'xpad{g}')
                nc.vector.memset(xp[:, :PADL], 0.0)
                nc.vector.memset(xp[:, PADL + L:], 0.0)
                x_pad.append(xp)
                zt = P.tile([128, L], BF16, tag='z', bufs=2, name=f'z{g}')
                z_t.append(zt)

            # stream h per L-half; x-part only (z deferred into AR window)
            def in_proj_part(obs, phase):
                for half in range(2):
                    h0 = half * (L // 2)
                    pps = {}
                    for ob in obs:
                        for c2 in range(2):
                            pps[(ob, c2)] = PS.tile(
                                [128, CH], F32, tag='bank', bufs=8,
                                name=f'pi{phase}{half}_{ob}_{c2}')
                    for k in range(NCORES):
                        hk = P.tile([128, L // 2], BF16, tag='hk', bufs=2,
                                    name=f'hk{phase}{half}_{k}')
                        nc.sync.dma_start(
                            out=hk[:],
                            in_=t['h_T'][k * 128:(k + 1) * 128,
                                         h0:h0 + L // 2])
                        for ob in obs:
                            col0 = ((ob % 2) * 128 if ob < 2
                                    else 256 + (ob % 2) * 128)
                            wk = P.tile([128, 128], BF16, tag='wstream',
                                        bufs=8, name=f'w{phase}{half}{ob}{k}')
                            nc.sync.dma_start(
                                out=wk[:],
                                in_=t['w_in_T'][k * 128:(k + 1) * 128,
                                                col0:col0 + 128])
                            for c2 in range(2):
                                nc.tensor.matmul(
                                    out=pps[(ob, c2)][:], lhsT=wk[:],
                                    rhs=hk[:, c2 * CH:(c2 + 1) * CH],
                                    start=(k == 0), stop=(k == NCORES - 1))
                    for ob in obs:
                        g = ob % 2
                        for c2 in range(2):
                            cc = h0 + c2 * CH
                            if ob < 2:
                                nc.scalar.activation(
                                    out=x_pad[g][:, PADL + cc:PADL + cc + CH],
                                    in_=pps[(ob, c2)][:], func=AF.Copy)
                            else:
                                nc.scalar.activation(
                                    out=z_t[g][:, cc:cc + CH],
                                    in_=pps[(ob, c2)][:], func=AF.Copy)

            in_proj_part([0, 1, 2, 3], 'a')

            # ---- conv + silu -> xa; Wx + AllReduce per direction ----
            xa = {}
            for d in range(2):
                for g in range(NG):
                    base = 0 if d == 0 else PADL
                    acc = P.tile([128, L], BF16, tag='cva', bufs=2,
                                 name=f'cv0_{d}{g}')
                    nc.vector.tensor_scalar(
                        out=acc[:], in0=x_pad[g][:, base:base + L],
                        scalar1=convw_g[g][:, 4 * d:4 * d + 1], scalar2=None,
                        op0=ALU.mult)
                    for k in range(1, KC):
                        acc2 = P.tile([128, L], BF16, tag='cva', bufs=2,
                                      name=f'cvk{k}_{d}{g}')
                        nc.vector.scalar_tensor_tensor(
                            out=acc2[:], in0=x_pad[g][:, base + k:base + k + L],
                            scalar=convw_g[g][:, 4 * d + k:4 * d + k + 1],
                            in1=acc[:], op0=ALU.mult, op1=ALU.add)
                        acc = acc2
                    xat = P.tile([128, L], BF16, tag='xa', bufs=4,
                                 name=f'xa{d}{g}')
                    nc.scalar.activation(out=xat[:], in_=acc[:], func=AF.Silu,
                                         bias=convb_g[g][:, d:d + 1])
                    xa[(d, g)] = xat
                xdbl = P.tile([96, L], BF16, tag='xdbl', bufs=1,
                              name=f'xdbl{d}')
                for c in range(TC4):
                    pw = PS.tile([128, CH], F32, tag='bank', bufs=8,
                                 name=f'pw{d}_{c}')
                    for g in range(NG):
                        nc.tensor.matmul(
                            out=pw[0:96, :],
                            lhsT=wx_g[g][:, 96 * d:96 * (d + 1)],
                            rhs=xa[(d, g)][:, c * CH:(c + 1) * CH],
                            start=(g == 0), stop=(g == NG - 1))
                    nc.scalar.activation(out=xdbl[:, c * CH:(c + 1) * CH],
                                         in_=pw[0:96, :], func=AF.Copy)
                nc.sync.dma_start(out=t[f'ar_in{d}'][:], in_=xdbl[:])
                nc.gpsimd.collective_compute(
                    'AllReduce', ALU.add,
                    replica_groups=[list(range(NCORES))],
                    ins=[t[f'ar_in{d}'][:]], outs=[t[f'ar_out{d}'][:]])

            # ---- dtraw + dt/dtx for all (g, d) ----
            dtraw = []
            for d in range(2):
                dr = P.tile([64, L], BF16, tag='dtraw', bufs=2, name=f'dtraw{d}')
                nc.sync.dma_start(out=dr[:], in_=t[f'ar_out{d}'][0:64, :])
                dtraw.append(dr)

            # ---- main scan loop: per-direction dt-proj + n-loop + drain ----
            dt_t = {}
            dtx_t = {}
            yd = {}
            py = None
            for d in range(2):
                for g in range(NG):
                    et = P.tile([128, L], BF16, tag='et', bufs=2,
                                name=f'et{g}{d}')
                    for c in range(TC4):
                        pd = PS.tile([128, CH], F32, tag='bank', bufs=8,
                                     name=f'pd{g}{d}{c}')
                        nc.tensor.matmul(
                            out=pd[:],
                            lhsT=wdt_t[:, d * DLOC + g * 128:
                                       d * DLOC + (g + 1) * 128],
                            rhs=dtraw[d][:, c * CH:(c + 1) * CH],
                            start=True, stop=True)
                        nc.scalar.activation(out=et[:, c * CH:(c + 1) * CH],
                                             in_=pd[:], func=AF.Exp,
                                             bias=bdt_g[g][:, d:d + 1])
                    dt = P.tile([128, L], BF16, tag='dt', bufs=4,
                                name=f'dt{g}{d}')
                    nc.scalar.activation(out=dt[:], in_=et[:], func=AF.Ln,
                                         bias=1.0)
                    dt_t[(g, d)] = dt
                    dtx = P.tile([128, L], BF16, tag='dtx', bufs=4,
                                 name=f'dtx{g}{d}')
                    nc.vector.tensor_tensor(out=dtx[:], in0=dt[:],
                                            in1=xa[(d, g)][:], op=ALU.mult)
                    dtx_t[(g, d)] = dtx

                py = [[PS.tile([128, CH], F32, tag='bank', bufs=8,
                               name=f'py{d}{g}_{c}') for c in range(TC4)]
                      for g in range(NG)]
                for n in range(N):
                    bbc = P.tile([128, L], BF16, tag='bbc', bufs=3,
                                 name=f'bbc{d}{n}')
                    nc.sync.dma_start(
                        out=bbc[:],
                        in_=t[f'ar_out{d}'][64 + n:64 + n + 1, :]
                        .to_broadcast([128, L]))
                    cbc = P.tile([128, L], BF16, tag='cbc', bufs=3,
                                 name=f'cbc{d}{n}')
                    nc.sync.dma_start(
                        out=cbc[:],
                        in_=t[f'ar_out{d}'][80 + n:80 + n + 1, :]
                        .to_broadcast([128, L]))
                    for g in range(NG):
                        dA = P.tile([128, L], F32, tag='dA', bufs=2,
                                    name=f'dA{d}{n}{g}')
                        nc.scalar.activation(
                            out=dA[:], in_=dt_t[(g, d)][:], func=AF.Exp,
                            scale=asc_g[g][:, d * N + n:d * N + n + 1])
                        dBx = P.tile([128, L], BF16, tag='dBx', bufs=3,
                                     name=f'dBx{d}{n}{g}')
                        nc.vector.tensor_tensor(
                            out=dBx[:], in0=dtx_t[(g, d)][:], in1=bbc[:],
                            op=ALU.mult)
                        h_t = P.tile([128, L], BF16, tag='h', bufs=3,
                                     name=f'h{d}{n}{g}')
                        if d == 0:
                            nc.vector.tensor_tensor_scan(
                                out=h_t[:], data0=dA[:], data1=dBx[:],
                                initial=0.0, op0=ALU.mult, op1=ALU.add)
                        else:
                            nc.vector.tensor_tensor_scan(
                                out=h_t[:, ::-1], data0=dA[:, ::-1],
                                data1=dBx[:, ::-1],
                                initial=0.0, op0=ALU.mult, op1=ALU.add)
                        hc = P.tile([128, L], BF16, tag='hc', bufs=3,
                                    name=f'hc{d}{n}{g}')
                        nc.vector.tensor_tensor(
                            out=hc[:], in0=h_t[:], in1=cbc[:], op=ALU.mult)
                        for c in range(TC4):
                            nc.tensor.matmul(
                                out=py[g][c][:], lhsT=ident[:],
                                rhs=hc[:, c * CH:(c + 1) * CH],
                                start=(n == 0), stop=(n == N - 1))
                if d == 0:
                    # drain direction-0 partial to SBUF, freeing PSUM banks
                    for g in range(NG):
                        ydt = P.tile([128, L], BF16, tag='yd', bufs=2,
                                     name=f'yd{g}')
                        for c in range(TC4):
                            nc.scalar.activation(
                                out=ydt[:, c * CH:(c + 1) * CH],
                                in_=py[g][c][:], func=AF.Copy)
                        yd[g] = ydt

            # ---- y tail ----
            yg_t = {}
            for g in range(NG):
                y1 = P.tile([128, L], BF16, tag='ytail', bufs=2, name=f'y1_{g}')
                for c in range(TC4):
                    nc.vector.scalar_tensor_tensor(
                        out=y1[:, c * CH:(c + 1) * CH],
                        in0=xa[(0, g)][:, c * CH:(c + 1) * CH],
                        scalar=d_g[g][:, 0:1],
                        in1=py[g][c][:], op0=ALU.mult, op1=ALU.add)
                y1b = P.tile([128, L], BF16, tag='ytail', bufs=2,
                             name=f'y1b_{g}')
                nc.vector.tensor_tensor(out=y1b[:], in0=y1[:], in1=yd[g][:],
                                        op=ALU.add)
                y2 = P.tile([128, L], BF16, tag='ytail', bufs=2, name=f'y2_{g}')
                nc.vector.scalar_tensor_tensor(
                    out=y2[:], in0=xa[(1, g)][:], scalar=d_g[g][:, 1:2],
                    in1=y1b[:], op0=ALU.mult, op1=ALU.add)
                sg = P.tile([128, L], BF16, tag='sg', bufs=1, name=f'sg{g}')
                nc.scalar.activation(out=sg[:], in_=z_t[g][:], func=AF.Silu)
                yg = P.tile([128, L], BF16, tag='yg', bufs=2, name=f'yg{g}')
                nc.vector.tensor_tensor(out=yg[:], in0=y2[:], in1=sg[:],
                                        op=ALU.mult)
                yg_t[g] = yg

            # ---- out_proj: two mb-halves, each followed by its own RS ----
            for hf in range(2):
                for mb in range(hf * 4, hf * 4 + 4):
                    wos = []
                    for g in range(NG):
                        wo = P.tile([128, 128], BF16, tag='wstream', bufs=8,
                                    name=f'wo{mb}_{g}')
                        nc.sync.dma_start(
                            out=wo[:],
                            in_=t['w_out_T'][g * 128:(g + 1) * 128,
                                             mb * 128:(mb + 1) * 128])
                        wos.append(wo)
                    for c in range(TC4):
                        po = PS.tile([128, CH], F32, tag='bank', bufs=8,
                                     name=f'po{mb}{c}')
                        for g in range(NG):
                            nc.tensor.matmul(
                                out=po[:], lhsT=wos[g][:],
                                rhs=yg_t[g][:, c * CH:(c + 1) * CH],
                                start=(g == 0), stop=(g == NG - 1))
                        ost = P.tile([128, CH], BF16, tag='ost', bufs=2,
                                     name=f'os{mb}{c}')
                        nc.scalar.activation(out=ost[:], in_=po[:],
                                             func=AF.Copy)
                        nc.sync.dma_start(
                            out=t[f'rs_in{hf}'][(mb - hf * 4) * 128:
                                                (mb - hf * 4 + 1) * 128,
                                                c * CH:(c + 1) * CH],
                            in_=ost[:])
                nc.gpsimd.collective_compute(
                    'ReduceScatter', ALU.add,
                    replica_groups=[list(range(NCORES))],
                    ins=[t[f'rs_in{hf}'][:]], outs=[t[f'rs_out{hf}'][:]])
                nc.sync.dma_start(out=t[f'out_bT{hf}'][:],
                                  in_=t[f'rs_out{hf}'][:])


def _build():
    nc = bacc.Bacc(None, target_bir_lowering=False)

    def inp(name, shape, dt=F32):
        return nc.declare_dram_parameter(name, shape, dt, isOutput=False)

    t = {
        'h_T': inp('h_T', [DM, L], BF16),
        'w_in_T': inp('w_in_T', [DM, 2 * DLOC], BF16),
        'wx_T': inp('wx_T', [DLOC, 2 * 96], BF16),
        'wdt_T': inp('wdt_T', [R, 2 * DLOC], BF16),
        'w_out_T': inp('w_out_T', [DLOC, DM], BF16),
        'bdt': inp('bdt', [DLOC, 2]),
        'conv_w': inp('conv_w', [DLOC, 2 * KC]),
        'conv_b': inp('conv_b', [DLOC, 2]),
        'd_both': inp('d_both', [DLOC, 2]),
        'a_sc': inp('a_sc', [DLOC, 2 * N]),
        'ident': inp('ident', [128, 128], BF16),
        'out_bT0': nc.declare_dram_parameter('out_bT0', [64, L], BF16,
                                             isOutput=True),
        'out_bT1': nc.declare_dram_parameter('out_bT1', [64, L], BF16,
                                             isOutput=True),
        'ar_in0': nc.dram_tensor('ar_in0', [96, L], BF16),
        'ar_in1': nc.dram_tensor('ar_in1', [96, L], BF16),
        'ar_out0': nc.dram_tensor('ar_out0', [96, L], BF16,
                                  addr_space='Shared'),
        'ar_out1': nc.dram_tensor('ar_out1', [96, L], BF16,
                                  addr_space='Shared'),
        'rs_in0': nc.dram_tensor('rs_in0', [DM // 2, L], BF16),
        'rs_in1': nc.dram_tensor('rs_in1', [DM // 2, L], BF16),
        'rs_out0': nc.dram_tensor('rs_out0', [64, L], BF16),
        'rs_out1': nc.dram_tensor('rs_out1', [64, L], BF16),
    }
    _emit(nc, t)
    nc.compile()
    return nc


def prepare_in_maps(inputs):
    import ml_dtypes
    f32 = np.float32
    bf16 = ml_dtypes.bfloat16
    h = np.asarray(inputs['hidden_states'], f32)[0]        # [L, DM]
    h_T = np.ascontiguousarray(h.T).astype(bf16)
    W_in = np.asarray(inputs['W_in'], f32)
    W_out = np.asarray(inputs['W_out'], f32)
    ident = np.eye(128, dtype=f32).astype(bf16)

    maps = []
    for c in range(NCORES):
        sl = slice(c * DLOC, (c + 1) * DLOC)
        cw_f = np.asarray(inputs['conv_w_fwd'], f32)[sl]          # natural taps
        cw_r = np.asarray(inputs['conv_w_rev'], f32)[sl][:, ::-1]  # flipped
        a_sc = np.concatenate(
            [-np.exp(np.asarray(inputs['A_log_fwd'], f32)[sl]),
             -np.exp(np.asarray(inputs['A_log_rev'], f32)[sl])], axis=1)
        m = {
            'h_T': h_T,
            'w_in_T': np.ascontiguousarray(np.concatenate(
                [W_in[sl].T, W_in[DI + c * DLOC:DI + (c + 1) * DLOC].T],
                axis=1)).astype(bf16),
            'wx_T': np.ascontiguousarray(np.concatenate(
                [np.asarray(inputs['Wx_fwd'], f32)[:, sl].T,
                 np.asarray(inputs['Wx_rev'], f32)[:, sl].T],
                axis=1)).astype(bf16),
            'wdt_T': np.ascontiguousarray(np.concatenate(
                [np.asarray(inputs['Wdt_fwd'], f32)[sl].T,
                 np.asarray(inputs['Wdt_rev'], f32)[sl].T],
                axis=1)).astype(bf16),
            'w_out_T': np.ascontiguousarray(W_out[:, sl].T).astype(bf16),
            'bdt': np.ascontiguousarray(np.stack(
                [np.asarray(inputs['bdt_fwd'], f32)[sl],
                 np.asarray(inputs['bdt_rev'], f32)[sl]], axis=1)),
            'conv_w': np.ascontiguousarray(
                np.concatenate([cw_f, cw_r], axis=1)),
            'conv_b': np.ascontiguousarray(np.stack(
                [np.asarray(inputs['conv_b_fwd'], f32)[sl],
                 np.asarray(inputs['conv_b_rev'], f32)[sl]], axis=1)),
            'd_both': np.ascontiguousarray(np.stack(
                [np.asarray(inputs['D_fwd'], f32)[sl],
                 np.asarray(inputs['D_rev'], f32)[sl]], axis=1)),
            'a_sc': np.ascontiguousarray(a_sc),
            'ident': ident,
        }
        maps.append(m)
    return maps


def get_nc():
    if 'nc' not in _CACHE:
        _CACHE['nc'] = _build()
    return _CACHE['nc']


def run(inputs, **kw):
    nc = get_nc()
    maps = prepare_in_maps(inputs)
    res = run_bass_kernel_spmd(nc, maps, list(range(NCORES)), **kw)
    out_T = np.empty((DM, L), np.float32)
    for c in range(NCORES):
        out_T[c * 64:(c + 1) * 64] = \
            np.asarray(res.results[c]['out_bT0']).astype(np.float32)
        out_T[DM // 2 + c * 64:DM // 2 + (c + 1) * 64] = \
            np.asarray(res.results[c]['out_bT1']).astype(np.float32)
    out = np.ascontiguousarray(out_T.T)[None]             # [1, L, DM]
    return out.astype(np.float32), res


def kernel(**inputs):
    out, _ = run(inputs)
    return out


# revision 16
# speedup vs baseline: 1.0637x; 1.0637x over previous
"""BiMamba (bidirectional Mamba v1) TRN2 Bass kernel, 8-core SPMD.

Shapes (hardcoded): B=1, L=2048, Dm=1024, Di=2048, N=16, K=4, R=64.
Sharding: tensor-parallel over d_inner across 8 cores (Di_loc=256).
Collectives: AllReduce of x_dbl (dt/B/C projections), ReduceScatter of out^T
(both in bf16).

Scan layout: per-(dir, state) tiles [128 channels, L].  dA comes straight
from Exp(dt * A[:, n]) with a per-partition scale AP; B/C rows are broadcast
from the AllReduce result in DRAM to all 128 partitions by cast-DMAs; the
n-contraction is an identity matmul accumulating into a PSUM py tile.  The
backward direction is the same scan with negative strides.
"""
import sys

sys.path.insert(0, '/opt/trn_rl_repo')

import numpy as np

import concourse.bacc as bacc
import concourse.mybir as mybir
from concourse.bass_utils import run_bass_kernel_spmd
from concourse.tile import TileContext

AF = mybir.ActivationFunctionType
ALU = mybir.AluOpType
F32 = mybir.dt.float32
BF16 = mybir.dt.bfloat16

L = 2048
DM = 1024
DI = 2048
N = 16
KC = 4
R = 64
NCORES = 8
DLOC = DI // NCORES      # 256
NG = DLOC // 128         # 2
CH = 512
TC4 = L // CH            # 4
PADL = KC - 1            # 3
XPW = L + 2 * PADL

_CACHE = {}


def _emit(nc, t):
    with TileContext(nc) as tc:
        with tc.tile_pool(name='sb', bufs=1) as P, \
             tc.tile_pool(name='ps', bufs=1, space='PSUM') as PS:

            # ---- small persistent loads ----
            def load2(name, src, cols, dt=F32):
                out = []
                for g in range(NG):
                    tl = P.tile([128, cols], dt, tag=name, bufs=2,
                                name=f'{name}{g}')
                    nc.sync.dma_start(out=tl[:],
                                      in_=src[g * 128:(g + 1) * 128, :])
                    out.append(tl)
                return out

            bdt_g = load2('bdt', t['bdt'], 2)            # fwd | rev
            convw_g = load2('convw', t['conv_w'], 2 * KC)
            convb_g = load2('convb', t['conv_b'], 2)
            d_g = load2('dboth', t['d_both'], 2)
            asc_g = load2('asc', t['a_sc'], 2 * N)       # [128, 32] f32
            ident = P.tile([128, 128], BF16, tag='ident', bufs=1, name='ident')
            nc.sync.dma_start(out=ident[:], in_=t['ident'][:])
            wx_g = []
            for g in range(NG):
                wxt = P.tile([128, 2 * 96], BF16, tag='wx', bufs=2,
                             name=f'wx{g}')
                nc.sync.dma_start(out=wxt[:],
                                  in_=t['wx_T'][g * 128:(g + 1) * 128, :])
                wx_g.append(wxt)
            wdt_t = P.tile([R, 2 * DLOC], BF16, tag='wdt', bufs=1, name='wdt_t')
            nc.sync.dma_start(out=wdt_t[:], in_=t['wdt_T'][:])

            # ---- in_proj ----
            x_pad = []
            z_t = []
            for g in range(NG):
                xp = P.tile([128, XPW], BF16, tag='xpad', bufs=2, name=f'xpad{g}')
                nc.vector.memset(xp[:, :PADL], 0.0)
                nc.vector.memset(xp[:, PADL + L:], 0.0)
                x_pad.append(xp)
                zt = P.tile([128, L], BF16, tag='z', bufs=2, name=f'z{g}')
                z_t.append(zt)

            # stream h per L-half; x-part only (z deferred into AR window)
            def in_proj_part(obs, phase):
                for half in range(2):
                    h0 = half * (L // 2)
                    pps = {}
                    for ob in obs:
                        for c2 in range(2):
                            pps[(ob, c2)] = PS.tile(
                                [128, CH], F32, tag='bank', bufs=8,
                                name=f'pi{phase}{half}_{ob}_{c2}')
                    for k in range(NCORES):
                        hk = P.tile([128, L // 2], BF16, tag='hk', bufs=3,
                                    name=f'hk{phase}{half}_{k}')
                        nc.sync.dma_start(
                            out=hk[:],
                            in_=t['h_T'][k * 128:(k + 1) * 128,
                                         h0:h0 + L // 2])
                        for ob in obs:
                            col0 = ((ob % 2) * 128 if ob < 2
                                    else 256 + (ob % 2) * 128)
                            wk = P.tile([128, 128], BF16, tag='wstream',
                                        bufs=8, name=f'w{phase}{half}{ob}{k}')
                            nc.gpsimd.dma_start(
                                out=wk[:],
                                in_=t['w_in_T'][k * 128:(k + 1) * 128,
                                                col0:col0 + 128])
                            for c2 in range(2):
                                nc.tensor.matmul(
                                    out=pps[(ob, c2)][:], lhsT=wk[:],
                                    rhs=hk[:, c2 * CH:(c2 + 1) * CH],
                                    start=(k == 0), stop=(k == NCORES - 1))
                    for ob in obs:
                        g = ob % 2
                        for c2 in range(2):
                            cc = h0 + c2 * CH
                            if ob < 2:
                                nc.scalar.activation(
                                    out=x_pad[g][:, PADL + cc:PADL + cc + CH],
                                    in_=pps[(ob, c2)][:], func=AF.Copy)
                            else:
                                nc.scalar.activation(
                                    out=z_t[g][:, cc:cc + CH],
                                    in_=pps[(ob, c2)][:], func=AF.Copy)

            in_proj_part([0, 1, 2, 3], 'a')

            # ---- conv + silu -> xa; Wx + AllReduce per direction ----
            xa = {}
            for d in range(2):
                for g in range(NG):
                    base = 0 if d == 0 else PADL
                    acc = P.tile([128, L], BF16, tag='cva', bufs=2,
                                 name=f'cv0_{d}{g}')
                    nc.vector.tensor_scalar(
                        out=acc[:], in0=x_pad[g][:, base:base + L],
                        scalar1=convw_g[g][:, 4 * d:4 * d + 1], scalar2=None,
                        op0=ALU.mult)
                    for k in range(1, KC):
                        acc2 = P.tile([128, L], BF16, tag='cva', bufs=2,
                                      name=f'cvk{k}_{d}{g}')
                        nc.vector.scalar_tensor_tensor(
                            out=acc2[:], in0=x_pad[g][:, base + k:base + k + L],
                            scalar=convw_g[g][:, 4 * d + k:4 * d + k + 1],
                            in1=acc[:], op0=ALU.mult, op1=ALU.add)
                        acc = acc2
                    xat = P.tile([128, L], BF16, tag='xa', bufs=4,
                                 name=f'xa{d}{g}')
                    nc.scalar.activation(out=xat[:], in_=acc[:], func=AF.Silu,
                                         bias=convb_g[g][:, d:d + 1])
                    xa[(d, g)] = xat
                xdbl = P.tile([96, L], BF16, tag='xdbl', bufs=2,
                              name=f'xdbl{d}')
                for c in range(TC4):
                    pw = PS.tile([128, CH], F32, tag='bank', bufs=8,
                                 name=f'pw{d}_{c}')
                    for g in range(NG):
                        nc.tensor.matmul(
                            out=pw[0:96, :],
                            lhsT=wx_g[g][:, 96 * d:96 * (d + 1)],
                            rhs=xa[(d, g)][:, c * CH:(c + 1) * CH],
                            start=(g == 0), stop=(g == NG - 1))
                    nc.scalar.activation(out=xdbl[:, c * CH:(c + 1) * CH],
                                         in_=pw[0:96, :], func=AF.Copy)
                nc.sync.dma_start(out=t[f'ar_in{d}'][:], in_=xdbl[:])
                nc.gpsimd.collective_compute(
                    'AllReduce', ALU.add,
                    replica_groups=[list(range(NCORES))],
                    ins=[t[f'ar_in{d}'][:]], outs=[t[f'ar_out{d}'][:]])

            # ---- dtraw + dt/dtx for all (g, d) ----
            dtraw = []
            for d in range(2):
                dr = P.tile([64, L], BF16, tag='dtraw', bufs=2, name=f'dtraw{d}')
                nc.sync.dma_start(out=dr[:], in_=t[f'ar_out{d}'][0:64, :])
                dtraw.append(dr)

            # ---- dt-proj for all (g, d) ----
            dt_t = {}
            dtx_t = {}
            for d in range(2):
                for g in range(NG):
                    et = P.tile([128, L], BF16, tag='et', bufs=2,
                                name=f'et{g}{d}')
                    for c in range(TC4):
                        pd = PS.tile([128, CH], F32, tag='bank', bufs=8,
                                     name=f'pd{g}{d}{c}')
                        nc.tensor.matmul(
                            out=pd[:],
                            lhsT=wdt_t[:, d * DLOC + g * 128:
                                       d * DLOC + (g + 1) * 128],
                            rhs=dtraw[d][:, c * CH:(c + 1) * CH],
                            start=True, stop=True)
                        nc.scalar.activation(out=et[:, c * CH:(c + 1) * CH],
                                             in_=pd[:], func=AF.Exp,
                                             bias=bdt_g[g][:, d:d + 1])
                    dt = P.tile([128, L], BF16, tag='dt', bufs=4,
                                name=f'dt{g}{d}')
                    nc.scalar.activation(out=dt[:], in_=et[:], func=AF.Ln,
                                         bias=1.0)
                    dt_t[(g, d)] = dt
                    dtx = P.tile([128, L], BF16, tag='dtx', bufs=4,
                                 name=f'dtx{g}{d}')
                    nc.vector.tensor_tensor(out=dtx[:], in0=dt[:],
                                            in1=xa[(d, g)][:], op=ALU.mult)
                    dtx_t[(g, d)] = dtx

            # ---- main scan loop: d -> n -> g; py per g in PSUM ----
            py = [[PS.tile([128, CH], F32, tag='bank', bufs=8,
                           name=f'py{g}_{c}') for c in range(TC4)]
                  for g in range(NG)]
            for d in range(2):
                for n in range(N):
                    bbc = P.tile([128, L], BF16, tag='bbc', bufs=3,
                                 name=f'bbc{d}{n}')
                    nc.sync.dma_start(
                        out=bbc[:],
                        in_=t[f'ar_out{d}'][64 + n:64 + n + 1, :]
                        .to_broadcast([128, L]))
                    cbc = P.tile([128, L], BF16, tag='cbc', bufs=3,
                                 name=f'cbc{d}{n}')
                    nc.sync.dma_start(
                        out=cbc[:],
                        in_=t[f'ar_out{d}'][80 + n:80 + n + 1, :]
                        .to_broadcast([128, L]))
                    for g in range(NG):
                        dA = P.tile([128, L], F32, tag='dA', bufs=2,
                                    name=f'dA{d}{n}{g}')
                        nc.scalar.activation(
                            out=dA[:], in_=dt_t[(g, d)][:], func=AF.Exp,
                            scale=asc_g[g][:, d * N + n:d * N + n + 1])
                        dBx = P.tile([128, L], BF16, tag='dBx', bufs=3,
                                     name=f'dBx{d}{n}{g}')
                        nc.vector.tensor_tensor(
                            out=dBx[:], in0=dtx_t[(g, d)][:], in1=bbc[:],
                            op=ALU.mult)
                        h_t = P.tile([128, L], BF16, tag='h', bufs=3,
                                     name=f'h{d}{n}{g}')
                        if d == 0:
                            nc.vector.tensor_tensor_scan(
                                out=h_t[:], data0=dA[:], data1=dBx[:],
                                initial=0.0, op0=ALU.mult, op1=ALU.add)
                        else:
                            nc.vector.tensor_tensor_scan(
                                out=h_t[:, ::-1], data0=dA[:, ::-1],
                                data1=dBx[:, ::-1],
                                initial=0.0, op0=ALU.mult, op1=ALU.add)
                        hc = P.tile([128, L], BF16, tag='hc', bufs=3,
                                    name=f'hc{d}{n}{g}')
                        nc.vector.tensor_tensor(
                            out=hc[:], in0=h_t[:], in1=cbc[:], op=ALU.mult)
                        for c in range(TC4):
                            nc.tensor.matmul(
                                out=py[g][c][:], lhsT=ident[:],
                                rhs=hc[:, c * CH:(c + 1) * CH],
                                start=(d == 0 and n == 0),
                                stop=(d == 1 and n == N - 1))

            # ---- y tail ----
            yg_t = {}
            for g in range(NG):
                y1 = P.tile([128, L], BF16, tag='ytail', bufs=2, name=f'y1_{g}')
                for c in range(TC4):
                    nc.vector.scalar_tensor_tensor(
                        out=y1[:, c * CH:(c + 1) * CH],
                        in0=xa[(0, g)][:, c * CH:(c + 1) * CH],
                        scalar=d_g[g][:, 0:1],
                        in1=py[g][c][:], op0=ALU.mult, op1=ALU.add)
                y2 = P.tile([128, L], BF16, tag='ytail', bufs=2, name=f'y2_{g}')
                nc.vector.scalar_tensor_tensor(
                    out=y2[:], in0=xa[(1, g)][:], scalar=d_g[g][:, 1:2],
                    in1=y1[:], op0=ALU.mult, op1=ALU.add)
                sg = P.tile([128, L], BF16, tag='sg', bufs=1, name=f'sg{g}')
                nc.scalar.activation(out=sg[:], in_=z_t[g][:], func=AF.Silu)
                yg = P.tile([128, L], BF16, tag='yg', bufs=2, name=f'yg{g}')
                nc.vector.tensor_tensor(out=yg[:], in0=y2[:], in1=sg[:],
                                        op=ALU.mult)
                yg_t[g] = yg

            # ---- out_proj: two mb-halves, each followed by its own RS ----
            for hf in range(2):
                for mb in range(hf * 4, hf * 4 + 4):
                    wos = []
                    for g in range(NG):
                        wo = P.tile([128, 128], BF16, tag='wstream', bufs=8,
                                    name=f'wo{mb}_{g}')
                        nc.gpsimd.dma_start(
                            out=wo[:],
                            in_=t['w_out_T'][g * 128:(g + 1) * 128,
                                             mb * 128:(mb + 1) * 128])
                        wos.append(wo)
                    for c in range(TC4):
                        po = PS.tile([128, CH], F32, tag='bank', bufs=8,
                                     name=f'po{mb}{c}')
                        for g in range(NG):
                            nc.tensor.matmul(
                                out=po[:], lhsT=wos[g][:],
                                rhs=yg_t[g][:, c * CH:(c + 1) * CH],
                                start=(g == 0), stop=(g == NG - 1))
                        ost = P.tile([128, CH], BF16, tag='ost', bufs=4,
                                     name=f'os{mb}{c}')
                        nc.scalar.activation(out=ost[:], in_=po[:],
                                             func=AF.Copy)
                        nc.sync.dma_start(
                            out=t[f'rs_in{hf}'][(mb - hf * 4) * 128:
                                                (mb - hf * 4 + 1) * 128,
                                                c * CH:(c + 1) * CH],
                            in_=ost[:])
                nc.gpsimd.collective_compute(
                    'ReduceScatter', ALU.add,
                    replica_groups=[list(range(NCORES))],
                    ins=[t[f'rs_in{hf}'][:]], outs=[t[f'rs_out{hf}'][:]])
                nc.sync.dma_start(out=t[f'out_bT{hf}'][:],
                                  in_=t[f'rs_out{hf}'][:])


def _build():
    nc = bacc.Bacc(None, target_bir_lowering=False)

    def inp(name, shape, dt=F32):
        return nc.declare_dram_parameter(name, shape, dt, isOutput=False)

    t = {
        'h_T': inp('h_T', [DM, L], BF16),
        'w_in_T': inp('w_in_T', [DM, 2 * DLOC], BF16),
        'wx_T': inp('wx_T', [DLOC, 2 * 96], BF16),
        'wdt_T': inp('wdt_T', [R, 2 * DLOC], BF16),
        'w_out_T': inp('w_out_T', [DLOC, DM], BF16),
        'bdt': inp('bdt', [DLOC, 2]),
        'conv_w': inp('conv_w', [DLOC, 2 * KC]),
        'conv_b': inp('conv_b', [DLOC, 2]),
        'd_both': inp('d_both', [DLOC, 2]),
        'a_sc': inp('a_sc', [DLOC, 2 * N]),
        'ident': inp('ident', [128, 128], BF16),
        'out_bT0': nc.declare_dram_parameter('out_bT0', [64, L], BF16,
                                             isOutput=True),
        'out_bT1': nc.declare_dram_parameter('out_bT1', [64, L], BF16,
                                             isOutput=True),
        'ar_in0': nc.dram_tensor('ar_in0', [96, L], BF16),
        'ar_in1': nc.dram_tensor('ar_in1', [96, L], BF16),
        'ar_out0': nc.dram_tensor('ar_out0', [96, L], BF16,
                                  addr_space='Shared'),
        'ar_out1': nc.dram_tensor('ar_out1', [96, L], BF16,
                                  addr_space='Shared'),
        'rs_in0': nc.dram_tensor('rs_in0', [DM // 2, L], BF16),
        'rs_in1': nc.dram_tensor('rs_in1', [DM // 2, L], BF16),
        'rs_out0': nc.dram_tensor('rs_out0', [64, L], BF16),
        'rs_out1': nc.dram_tensor('rs_out1', [64, L], BF16),
    }
    _emit(nc, t)
    nc.compile()
    return nc


def prepare_in_maps(inputs):
    import ml_dtypes
    f32 = np.float32
    bf16 = ml_dtypes.bfloat16
    h = np.asarray(inputs['hidden_states'], f32)[0]        # [L, DM]
    h_T = np.ascontiguousarray(h.T).astype(bf16)
    W_in = np.asarray(inputs['W_in'], f32)
    W_out = np.asarray(inputs['W_out'], f32)
    ident = np.eye(128, dtype=f32).astype(bf16)

    maps = []
    for c in range(NCORES):
        sl = slice(c * DLOC, (c + 1) * DLOC)
        cw_f = np.asarray(inputs['conv_w_fwd'], f32)[sl]          # natural taps
        cw_r = np.asarray(inputs['conv_w_rev'], f32)[sl][:, ::-1]  # flipped
        a_sc = np.concatenate(
            [-np.exp(np.asarray(inputs['A_log_fwd'], f32)[sl]),
             -np.exp(np.asarray(inputs['A_log_rev'], f32)[sl])], axis=1)
        m = {
            'h_T': h_T,
            'w_in_T': np.ascontiguousarray(np.concatenate(
                [W_in[sl].T, W_in[DI + c * DLOC:DI + (c + 1) * DLOC].T],
                axis=1)).astype(bf16),
            'wx_T': np.ascontiguousarray(np.concatenate(
                [np.asarray(inputs['Wx_fwd'], f32)[:, sl].T,
                 np.asarray(inputs['Wx_rev'], f32)[:, sl].T],
                axis=1)).astype(bf16),
            'wdt_T': np.ascontiguousarray(np.concatenate(
                [np.asarray(inputs['Wdt_fwd'], f32)[sl].T,
                 np.asarray(inputs['Wdt_rev'], f32)[sl].T],
                axis=1)).astype(bf16),
            'w_out_T': np.ascontiguousarray(W_out[:, sl].T).astype(bf16),
            'bdt': np.ascontiguousarray(np.stack(
                [np.asarray(inputs['bdt_fwd'], f32)[sl],
                 np.asarray(inputs['bdt_rev'], f32)[sl]], axis=1)),
            'conv_w': np.ascontiguousarray(
                np.concatenate([cw_f, cw_r], axis=1)),
            'conv_b': np.ascontiguousarray(np.stack(
                [np.asarray(inputs['conv_b_fwd'], f32)[sl],
                 np.asarray(inputs['conv_b_rev'], f32)[sl]], axis=1)),
            'd_both': np.ascontiguousarray(np.stack(
                [np.asarray(inputs['D_fwd'], f32)[sl],
                 np.asarray(inputs['D_rev'], f32)[sl]], axis=1)),
            'a_sc': np.ascontiguousarray(a_sc),
            'ident': ident,
        }
        maps.append(m)
    return maps


def get_nc():
    if 'nc' not in _CACHE:
        _CACHE['nc'] = _build()
    return _CACHE['nc']


def run(inputs, **kw):
    nc = get_nc()
    maps = prepare_in_maps(inputs)
    res = run_bass_kernel_spmd(nc, maps, list(range(NCORES)), **kw)
    out_T = np.empty((DM, L), np.float32)
    for c in range(NCORES):
        out_T[c * 64:(c + 1) * 64] = \
            np.asarray(res.results[c]['out_bT0']).astype(np.float32)
        out_T[DM // 2 + c * 64:DM // 2 + (c + 1) * 64] = \
            np.asarray(res.results[c]['out_bT1']).astype(np.float32)
    out = np.ascontiguousarray(out_T.T)[None]             # [1, L, DM]
    return out.astype(np.float32), res


def kernel(**inputs):
    out, _ = run(inputs)
    return out


# revision 19
# speedup vs baseline: 1.0641x; 1.0004x over previous
"""BiMamba (bidirectional Mamba v1) TRN2 Bass kernel, 8-core SPMD.

Shapes (hardcoded): B=1, L=2048, Dm=1024, Di=2048, N=16, K=4, R=64.
Sharding: tensor-parallel over d_inner across 8 cores (Di_loc=256).
Collectives: AllReduce of x_dbl (dt/B/C projections), ReduceScatter of out^T
(both in bf16).

Scan layout: per-(dir, state) tiles [128 channels, L].  dA comes straight
from Exp(dt * A[:, n]) with a per-partition scale AP; B/C rows are broadcast
from the AllReduce result in DRAM to all 128 partitions by cast-DMAs; the
n-contraction is an identity matmul accumulating into a PSUM py tile.  The
backward direction is the same scan with negative strides.
"""
import sys

sys.path.insert(0, '/opt/trn_rl_repo')

import numpy as np

import concourse.bacc as bacc
import concourse.mybir as mybir
from concourse.bass_utils import run_bass_kernel_spmd
from concourse.tile import TileContext

AF = mybir.ActivationFunctionType
ALU = mybir.AluOpType
F32 = mybir.dt.float32
BF16 = mybir.dt.bfloat16

L = 2048
DM = 1024
DI = 2048
N = 16
KC = 4
R = 64
NCORES = 8
DLOC = DI // NCORES      # 256
NG = DLOC // 128         # 2
CH = 512
TC4 = L // CH            # 4
PADL = KC - 1            # 3
XPW = L + 2 * PADL

_CACHE = {}


def _emit(nc, t):
    with TileContext(nc) as tc:
        with tc.tile_pool(name='sb', bufs=1) as P, \
             tc.tile_pool(name='ps', bufs=1, space='PSUM') as PS:

            # ---- small persistent loads ----
            def load2(name, src, cols, dt=F32):
                out = []
                for g in range(NG):
                    tl = P.tile([128, cols], dt, tag=name, bufs=2,
                                name=f'{name}{g}')
                    nc.sync.dma_start(out=tl[:],
                                      in_=src[g * 128:(g + 1) * 128, :])
                    out.append(tl)
                return out

            bdt_g = load2('bdt', t['bdt'], 2)            # fwd | rev
            convw_g = load2('convw', t['conv_w'], 2 * KC)
            convb_g = load2('convb', t['conv_b'], 2)
            d_g = load2('dboth', t['d_both'], 2)
            asc_g = load2('asc', t['a_sc'], 2 * N)       # [128, 32] f32
            ident = P.tile([128, 128], BF16, tag='ident', bufs=1, name='ident')
            nc.sync.dma_start(out=ident[:], in_=t['ident'][:])
            wx_g = []
            for g in range(NG):
                wxt = P.tile([128, 2 * 96], BF16, tag='wx', bufs=2,
                             name=f'wx{g}')
                nc.sync.dma_start(out=wxt[:],
                                  in_=t['wx_T'][g * 128:(g + 1) * 128, :])
                wx_g.append(wxt)
            wdt_t = P.tile([R, 2 * DLOC], BF16, tag='wdt', bufs=1, name='wdt_t')
            nc.sync.dma_start(out=wdt_t[:], in_=t['wdt_T'][:])

            # ---- in_proj ----
            x_pad = []
            z_t = []
            for g in range(NG):
                xp = P.tile([128, XPW], BF16, tag='xpad', bufs=2, name=f'xpad{g}')
                nc.vector.memset(xp[:, :PADL], 0.0)
                nc.vector.memset(xp[:, PADL + L:], 0.0)
                x_pad.append(xp)
                zt = P.tile([128, L], BF16, tag='z', bufs=2, name=f'z{g}')
                z_t.append(zt)

            # stream h per L-half; batched loads: 1 h-DMA per half, 1 W-DMA per ob
            def in_proj_part(obs, phase):
                for half in range(2):
                    h0 = half * (L // 2)
                    pps = {}
                    for ob in obs:
                        for c2 in range(2):
                            pps[(ob, c2)] = PS.tile(
                                [128, CH], F32, tag='bank', bufs=8,
                                name=f'pi{phase}{half}_{ob}_{c2}')
                    wts = {}
                    for ob in obs:
                        col0 = ((ob % 2) * 128 if ob < 2
                                else 256 + (ob % 2) * 128)
                        wt = P.tile([128, NCORES, 128], BF16, tag='wbatch',
                                    bufs=4, name=f'wb{phase}{half}{ob}')
                        nc.gpsimd.dma_start(
                            out=wt[:],
                            in_=t['w_in_T'][:].rearrange(
                                '(k p) c -> p k c', k=NCORES,
                                p=128)[:, :, col0:col0 + 128])
                        wts[ob] = wt
                    for k in range(NCORES):
                        hk = P.tile([128, L // 2], BF16, tag='hk', bufs=2,
                                    name=f'hk{phase}{half}_{k}')
                        nc.sync.dma_start(
                            out=hk[:],
                            in_=t['h_T'][k * 128:(k + 1) * 128,
                                         h0:h0 + L // 2])
                        for ob in obs:
                            for c2 in range(2):
                                nc.tensor.matmul(
                                    out=pps[(ob, c2)][:],
                                    lhsT=wts[ob][:, k, :],
                                    rhs=hk[:, c2 * CH:(c2 + 1) * CH],
                                    start=(k == 0), stop=(k == NCORES - 1))
                    for ob in obs:
                        g = ob % 2
                        for c2 in range(2):
                            cc = h0 + c2 * CH
                            if ob < 2:
                                nc.scalar.activation(
                                    out=x_pad[g][:, PADL + cc:PADL + cc + CH],
                                    in_=pps[(ob, c2)][:], func=AF.Copy)
                            else:
                                nc.scalar.activation(
                                    out=z_t[g][:, cc:cc + CH],
                                    in_=pps[(ob, c2)][:], func=AF.Copy)

            in_proj_part([0, 1, 2, 3], 'a')

            # ---- conv + silu -> xa; Wx + AllReduce per direction ----
            xa = {}
            for d in range(2):
                for g in range(NG):
                    base = 0 if d == 0 else PADL
                    acc = P.tile([128, L], BF16, tag='cva', bufs=2,
                                 name=f'cv0_{d}{g}')
                    nc.vector.tensor_scalar(
                        out=acc[:], in0=x_pad[g][:, base:base + L],
                        scalar1=convw_g[g][:, 4 * d:4 * d + 1], scalar2=None,
                        op0=ALU.mult)
                    for k in range(1, KC):
                        acc2 = P.tile([128, L], BF16, tag='cva', bufs=2,
                                      name=f'cvk{k}_{d}{g}')
                        nc.vector.scalar_tensor_tensor(
                            out=acc2[:], in0=x_pad[g][:, base + k:base + k + L],
                            scalar=convw_g[g][:, 4 * d + k:4 * d + k + 1],
                            in1=acc[:], op0=ALU.mult, op1=ALU.add)
                        acc = acc2
                    xat = P.tile([128, L], BF16, tag='xa', bufs=4,
                                 name=f'xa{d}{g}')
                    nc.scalar.activation(out=xat[:], in_=acc[:], func=AF.Silu,
                                         bias=convb_g[g][:, d:d + 1])
                    xa[(d, g)] = xat
                xdbl = P.tile([96, L], BF16, tag='xdbl', bufs=2,
                              name=f'xdbl{d}')
                for c in range(TC4):
                    pw = PS.tile([128, CH], F32, tag='bank', bufs=8,
                                 name=f'pw{d}_{c}')
                    for g in range(NG):
                        nc.tensor.matmul(
                            out=pw[0:96, :],
                            lhsT=wx_g[g][:, 96 * d:96 * (d + 1)],
                            rhs=xa[(d, g)][:, c * CH:(c + 1) * CH],
                            start=(g == 0), stop=(g == NG - 1))
                    nc.scalar.activation(out=xdbl[:, c * CH:(c + 1) * CH],
                                         in_=pw[0:96, :], func=AF.Copy)
                nc.sync.dma_start(out=t[f'ar_in{d}'][:], in_=xdbl[:])
                nc.gpsimd.collective_compute(
                    'AllReduce', ALU.add,
                    replica_groups=[list(range(NCORES))],
                    ins=[t[f'ar_in{d}'][:]], outs=[t[f'ar_out{d}'][:]])

            # ---- dtraw + dt/dtx for all (g, d) ----
            dtraw = []
            for d in range(2):
                dr = P.tile([64, L], BF16, tag='dtraw', bufs=2, name=f'dtraw{d}')
                nc.sync.dma_start(out=dr[:], in_=t[f'ar_out{d}'][0:64, :])
                dtraw.append(dr)

            # ---- dt-proj for all (g, d) ----
            dt_t = {}
            dtx_t = {}
            for d in range(2):
                for g in range(NG):
                    et = P.tile([128, L], BF16, tag='et', bufs=1,
                                name=f'et{g}{d}')
                    for c in range(TC4):
                        pd = PS.tile([128, CH], F32, tag='bank', bufs=8,
                                     name=f'pd{g}{d}{c}')
                        nc.tensor.matmul(
                            out=pd[:],
                            lhsT=wdt_t[:, d * DLOC + g * 128:
                                       d * DLOC + (g + 1) * 128],
                            rhs=dtraw[d][:, c * CH:(c + 1) * CH],
                            start=True, stop=True)
                        nc.scalar.activation(out=et[:, c * CH:(c + 1) * CH],
                                             in_=pd[:], func=AF.Exp,
                                             bias=bdt_g[g][:, d:d + 1])
                    dt = P.tile([128, L], BF16, tag='dt', bufs=4,
                                name=f'dt{g}{d}')
                    nc.scalar.activation(out=dt[:], in_=et[:], func=AF.Ln,
                                         bias=1.0)
                    dt_t[(g, d)] = dt
                    dtx = P.tile([128, L], BF16, tag='dtx', bufs=4,
                                 name=f'dtx{g}{d}')
                    nc.vector.tensor_tensor(out=dtx[:], in0=dt[:],
                                            in1=xa[(d, g)][:], op=ALU.mult)
                    dtx_t[(g, d)] = dtx

            # ---- main scan loop: d -> n -> g; py per g in PSUM ----
            py = [[PS.tile([128, CH], F32, tag='bank', bufs=8,
                           name=f'py{g}_{c}') for c in range(TC4)]
                  for g in range(NG)]
            for d in range(2):
                for n in range(N):
                    bc2 = P.tile([128, 2, L], BF16, tag='bc2', bufs=3,
                                 name=f'bc2{d}{n}')
                    src = t[f'ar_out{d}'][64 + n::16, :][0:2, :]
                    nc.sync.dma_start(
                        out=bc2[:],
                        in_=src.unsqueeze(0).to_broadcast([128, 2, L]))
                    bbc = bc2[:, 0, :]
                    cbc = bc2[:, 1, :]
                    for g in range(NG):
                        dA = P.tile([128, L], F32, tag='dA', bufs=2,
                                    name=f'dA{d}{n}{g}')
                        nc.scalar.activation(
                            out=dA[:], in_=dt_t[(g, d)][:], func=AF.Exp,
                            scale=asc_g[g][:, d * N + n:d * N + n + 1])
                        dBx = P.tile([128, L], BF16, tag='dBx', bufs=2,
                                     name=f'dBx{d}{n}{g}')
                        nc.vector.tensor_tensor(
                            out=dBx[:], in0=dtx_t[(g, d)][:], in1=bbc[:],
                            op=ALU.mult)
                        h_t = P.tile([128, L], BF16, tag='h', bufs=3,
                                     name=f'h{d}{n}{g}')
                        if d == 0:
                            nc.vector.tensor_tensor_scan(
                                out=h_t[:], data0=dA[:], data1=dBx[:],
                                initial=0.0, op0=ALU.mult, op1=ALU.add)
                        else:
                            nc.vector.tensor_tensor_scan(
                                out=h_t[:, ::-1], data0=dA[:, ::-1],
                                data1=dBx[:, ::-1],
                                initial=0.0, op0=ALU.mult, op1=ALU.add)
                        hc = P.tile([128, L], BF16, tag='hc', bufs=3,
                                    name=f'hc{d}{n}{g}')
                        nc.vector.tensor_tensor(
                            out=hc[:], in0=h_t[:], in1=cbc[:], op=ALU.mult)
                        for c in range(TC4):
                            nc.tensor.matmul(
                                out=py[g][c][:], lhsT=ident[:],
                                rhs=hc[:, c * CH:(c + 1) * CH],
                                start=(d == 0 and n == 0),
                                stop=(d == 1 and n == N - 1))

            # ---- y tail ----
            yg_t = {}
            for g in range(NG):
                y1 = P.tile([128, L], BF16, tag='ytail', bufs=2, name=f'y1_{g}')
                for c in range(TC4):
                    nc.vector.scalar_tensor_tensor(
                        out=y1[:, c * CH:(c + 1) * CH],
                        in0=xa[(0, g)][:, c * CH:(c + 1) * CH],
                        scalar=d_g[g][:, 0:1],
                        in1=py[g][c][:], op0=ALU.mult, op1=ALU.add)
                y2 = P.tile([128, L], BF16, tag='ytail', bufs=2, name=f'y2_{g}')
                nc.vector.scalar_tensor_tensor(
                    out=y2[:], in0=xa[(1, g)][:], scalar=d_g[g][:, 1:2],
                    in1=y1[:], op0=ALU.mult, op1=ALU.add)
                sg = P.tile([128, L], BF16, tag='sg', bufs=1, name=f'sg{g}')
                nc.scalar.activation(out=sg[:], in_=z_t[g][:], func=AF.Silu)
                yg = P.tile([128, L], BF16, tag='yg', bufs=2, name=f'yg{g}')
                nc.vector.tensor_tensor(out=yg[:], in0=y2[:], in1=sg[:],
                                        op=ALU.mult)
                yg_t[g] = yg

            # ---- out_proj: two mb-halves, each followed by its own RS ----
            for hf in range(2):
                for mb in range(hf * 4, hf * 4 + 4):
                    wos = []
                    for g in range(NG):
                        wo = P.tile([128, 128], BF16, tag='wstream', bufs=8,
                                    name=f'wo{mb}_{g}')
                        nc.gpsimd.dma_start(
                            out=wo[:],
                            in_=t['w_out_T'][g * 128:(g + 1) * 128,
                                             mb * 128:(mb + 1) * 128])
                        wos.append(wo)
                    for c in range(TC4):
                        po = PS.tile([128, CH], F32, tag='bank', bufs=8,
                                     name=f'po{mb}{c}')
                        for g in range(NG):
                            nc.tensor.matmul(
                                out=po[:], lhsT=wos[g][:],
                                rhs=yg_t[g][:, c * CH:(c + 1) * CH],
                                start=(g == 0), stop=(g == NG - 1))
                        ost = P.tile([128, CH], BF16, tag='ost', bufs=4,
                                     name=f'os{mb}{c}')
                        nc.scalar.activation(out=ost[:], in_=po[:],
                                             func=AF.Copy)
                        nc.sync.dma_start(
                            out=t[f'rs_in{hf}'][(mb - hf * 4) * 128:
                                                (mb - hf * 4 + 1) * 128,
                                                c * CH:(c + 1) * CH],
                            in_=ost[:])
                nc.gpsimd.collective_compute(
                    'ReduceScatter', ALU.add,
                    replica_groups=[list(range(NCORES))],
                    ins=[t[f'rs_in{hf}'][:]], outs=[t[f'rs_out{hf}'][:]])
                nc.sync.dma_start(out=t[f'out_bT{hf}'][:],
                                  in_=t[f'rs_out{hf}'][:])


def _build():
    nc = bacc.Bacc(None, target_bir_lowering=False)

    def inp(name, shape, dt=F32):
        return nc.declare_dram_parameter(name, shape, dt, isOutput=False)

    t = {
        'h_T': inp('h_T', [DM, L], BF16),
        'w_in_T': inp('w_in_T', [DM, 2 * DLOC], BF16),
        'wx_T': inp('wx_T', [DLOC, 2 * 96], BF16),
        'wdt_T': inp('wdt_T', [R, 2 * DLOC], BF16),
        'w_out_T': inp('w_out_T', [DLOC, DM], BF16),
        'bdt': inp('bdt', [DLOC, 2]),
        'conv_w': inp('conv_w', [DLOC, 2 * KC]),
        'conv_b': inp('conv_b', [DLOC, 2]),
        'd_both': inp('d_both', [DLOC, 2]),
        'a_sc': inp('a_sc', [DLOC, 2 * N]),
        'ident': inp('ident', [128, 128], BF16),
        'out_bT0': nc.declare_dram_parameter('out_bT0', [64, L], BF16,
                                             isOutput=True),
        'out_bT1': nc.declare_dram_parameter('out_bT1', [64, L], BF16,
                                             isOutput=True),
        'ar_in0': nc.dram_tensor('ar_in0', [96, L], BF16),
        'ar_in1': nc.dram_tensor('ar_in1', [96, L], BF16),
        'ar_out0': nc.dram_tensor('ar_out0', [96, L], BF16,
                                  addr_space='Shared'),
        'ar_out1': nc.dram_tensor('ar_out1', [96, L], BF16,
                                  addr_space='Shared'),
        'rs_in0': nc.dram_tensor('rs_in0', [DM // 2, L], BF16),
        'rs_in1': nc.dram_tensor('rs_in1', [DM // 2, L], BF16),
        'rs_out0': nc.dram_tensor('rs_out0', [64, L], BF16),
        'rs_out1': nc.dram_tensor('rs_out1', [64, L], BF16),
    }
    _emit(nc, t)
    nc.compile()
    return nc


def prepare_in_maps(inputs):
    import ml_dtypes
    f32 = np.float32
    bf16 = ml_dtypes.bfloat16
    h = np.asarray(inputs['hidden_states'], f32)[0]        # [L, DM]
    h_T = np.ascontiguousarray(h.T).astype(bf16)
    W_in = np.asarray(inputs['W_in'], f32)
    W_out = np.asarray(inputs['W_out'], f32)
    ident = np.eye(128, dtype=f32).astype(bf16)

    maps = []
    for c in range(NCORES):
        sl = slice(c * DLOC, (c + 1) * DLOC)
        cw_f = np.asarray(inputs['conv_w_fwd'], f32)[sl]          # natural taps
        cw_r = np.asarray(inputs['conv_w_rev'], f32)[sl][:, ::-1]  # flipped
        a_sc = np.concatenate(
            [-np.exp(np.asarray(inputs['A_log_fwd'], f32)[sl]),
             -np.exp(np.asarray(inputs['A_log_rev'], f32)[sl])], axis=1)
        m = {
            'h_T': h_T,
            'w_in_T': np.ascontiguousarray(np.concatenate(
                [W_in[sl].T, W_in[DI + c * DLOC:DI + (c + 1) * DLOC].T],
                axis=1)).astype(bf16),
            'wx_T': np.ascontiguousarray(np.concatenate(
                [np.asarray(inputs['Wx_fwd'], f32)[:, sl].T,
                 np.asarray(inputs['Wx_rev'], f32)[:, sl].T],
                axis=1)).astype(bf16),
            'wdt_T': np.ascontiguousarray(np.concatenate(
                [np.asarray(inputs['Wdt_fwd'], f32)[sl].T,
                 np.asarray(inputs['Wdt_rev'], f32)[sl].T],
                axis=1)).astype(bf16),
            'w_out_T': np.ascontiguousarray(W_out[:, sl].T).astype(bf16),
            'bdt': np.ascontiguousarray(np.stack(
                [np.asarray(inputs['bdt_fwd'], f32)[sl],
                 np.asarray(inputs['bdt_rev'], f32)[sl]], axis=1)),
            'conv_w': np.ascontiguousarray(
                np.concatenate([cw_f, cw_r], axis=1)),
            'conv_b': np.ascontiguousarray(np.stack(
                [np.asarray(inputs['conv_b_fwd'], f32)[sl],
                 np.asarray(inputs['conv_b_rev'], f32)[sl]], axis=1)),
            'd_both': np.ascontiguousarray(np.stack(
                [np.asarray(inputs['D_fwd'], f32)[sl],
                 np.asarray(inputs['D_rev'], f32)[sl]], axis=1)),
            'a_sc': np.ascontiguousarray(a_sc),
            'ident': ident,
        }
        maps.append(m)
    return maps


def get_nc():
    if 'nc' not in _CACHE:
        _CACHE['nc'] = _build()
    return _CACHE['nc']


def run(inputs, **kw):
    nc = get_nc()
    maps = prepare_in_maps(inputs)
    res = run_bass_kernel_spmd(nc, maps, list(range(NCORES)), **kw)
    out_T = np.empty((DM, L), np.float32)
    for c in range(NCORES):
        out_T[c * 64:(c + 1) * 64] = \
            np.asarray(res.results[c]['out_bT0']).astype(np.float32)
        out_T[DM // 2 + c * 64:DM // 2 + (c + 1) * 64] = \
            np.asarray(res.results[c]['out_bT1']).astype(np.float32)
    out = np.ascontiguousarray(out_T.T)[None]             # [1, L, DM]
    return out.astype(np.float32), res


def kernel(**inputs):
    out, _ = run(inputs)
    return out


# revision 20
# speedup vs baseline: 1.0819x; 1.0167x over previous
"""BiMamba (bidirectional Mamba v1) TRN2 Bass kernel, 8-core SPMD.

Shapes (hardcoded): B=1, L=2048, Dm=1024, Di=2048, N=16, K=4, R=64.
Sharding: tensor-parallel over d_inner across 8 cores (Di_loc=256).
Collectives: AllReduce of x_dbl (dt/B/C projections), ReduceScatter of out^T
(both in bf16).

Scan layout: per-(dir, state) tiles [128 channels, L].  dA comes straight
from Exp(dt * A[:, n]) with a per-partition scale AP; B/C rows are broadcast
from the AllReduce result in DRAM to all 128 partitions by cast-DMAs; the
n-contraction is an identity matmul accumulating into a PSUM py tile.  The
backward direction is the same scan with negative strides.
"""
import sys

sys.path.insert(0, '/opt/trn_rl_repo')

import numpy as np

import concourse.bacc as bacc
import concourse.mybir as mybir
from concourse.bass_utils import run_bass_kernel_spmd
from concourse.tile import TileContext

AF = mybir.ActivationFunctionType
ALU = mybir.AluOpType
F32 = mybir.dt.float32
BF16 = mybir.dt.bfloat16

L = 2048
DM = 1024
DI = 2048
N = 16
KC = 4
R = 64
NCORES = 8
DLOC = DI // NCORES      # 256
NG = DLOC // 128         # 2
CH = 512
TC4 = L // CH            # 4
PADL = KC - 1            # 3
XPW = L + 2 * PADL

_CACHE = {}


def _emit(nc, t):
    with TileContext(nc) as tc:
        with tc.tile_pool(name='sb', bufs=1) as P, \
             tc.tile_pool(name='ps', bufs=1, space='PSUM') as PS:

            # ---- small persistent loads ----
            def load2(name, src, cols, dt=F32):
                out = []
                for g in range(NG):
                    tl = P.tile([128, cols], dt, tag=name, bufs=2,
                                name=f'{name}{g}')
                    nc.sync.dma_start(out=tl[:],
                                      in_=src[g * 128:(g + 1) * 128, :])
                    out.append(tl)
                return out

            bdt_g = load2('bdt', t['bdt'], 2)            # fwd | rev
            convw_g = load2('convw', t['conv_w'], 2 * KC)
            convb_g = load2('convb', t['conv_b'], 2)
            d_g = load2('dboth', t['d_both'], 2)
            asc_g = load2('asc', t['a_sc'], 2 * N)       # [128, 32] f32
            ident = P.tile([128, 128], BF16, tag='ident', bufs=1, name='ident')
            nc.sync.dma_start(out=ident[:], in_=t['ident'][:])
            diag_d = {}
            for d in range(2):
                for g in range(NG):
                    dg = P.tile([128, 128], BF16, tag='diagd', bufs=4,
                                name=f'diagd{d}{g}')
                    nc.sync.dma_start(
                        out=dg[:],
                        in_=t['diag_d'][(2 * d + g) * 128:
                                        (2 * d + g + 1) * 128, :])
                    diag_d[(d, g)] = dg
            wx_g = []
            for g in range(NG):
                wxt = P.tile([128, 2 * 96], BF16, tag='wx', bufs=2,
                             name=f'wx{g}')
                nc.sync.dma_start(out=wxt[:],
                                  in_=t['wx_T'][g * 128:(g + 1) * 128, :])
                wx_g.append(wxt)
            wdt_t = P.tile([R, 2 * DLOC], BF16, tag='wdt', bufs=1, name='wdt_t')
            nc.sync.dma_start(out=wdt_t[:], in_=t['wdt_T'][:])

            # ---- in_proj ----
            x_pad = []
            z_t = []
            for g in range(NG):
                xp = P.tile([128, XPW], BF16, tag='xpad', bufs=2, name=f'xpad{g}')
                nc.vector.memset(xp[:, :PADL], 0.0)
                nc.vector.memset(xp[:, PADL + L:], 0.0)
                x_pad.append(xp)
                zt = P.tile([128, L], BF16, tag='z', bufs=2, name=f'z{g}')
                z_t.append(zt)

            # stream h per L-half; batched loads: 1 h-DMA per half, 1 W-DMA per ob
            def in_proj_part(obs, phase):
                for half in range(2):
                    h0 = half * (L // 2)
                    pps = {}
                    for ob in obs:
                        for c2 in range(2):
                            pps[(ob, c2)] = PS.tile(
                                [128, CH], F32, tag='bank', bufs=8,
                                name=f'pi{phase}{half}_{ob}_{c2}')
                    wts = {}
                    for ob in obs:
                        col0 = ((ob % 2) * 128 if ob < 2
                                else 256 + (ob % 2) * 128)
                        wt = P.tile([128, NCORES, 128], BF16, tag='wbatch',
                                    bufs=4, name=f'wb{phase}{half}{ob}')
                        nc.gpsimd.dma_start(
                            out=wt[:],
                            in_=t['w_in_T'][:].rearrange(
                                '(k p) c -> p k c', k=NCORES,
                                p=128)[:, :, col0:col0 + 128])
                        wts[ob] = wt
                    for k in range(NCORES):
                        hk = P.tile([128, L // 2], BF16, tag='hk', bufs=3,
                                    name=f'hk{phase}{half}_{k}')
                        nc.sync.dma_start(
                            out=hk[:],
                            in_=t['h_T'][k * 128:(k + 1) * 128,
                                         h0:h0 + L // 2])
                        for ob in obs:
                            for c2 in range(2):
                                nc.tensor.matmul(
                                    out=pps[(ob, c2)][:],
                                    lhsT=wts[ob][:, k, :],
                                    rhs=hk[:, c2 * CH:(c2 + 1) * CH],
                                    start=(k == 0), stop=(k == NCORES - 1))
                    for ob in obs:
                        g = ob % 2
                        for c2 in range(2):
                            cc = h0 + c2 * CH
                            if ob < 2:
                                nc.scalar.activation(
                                    out=x_pad[g][:, PADL + cc:PADL + cc + CH],
                                    in_=pps[(ob, c2)][:], func=AF.Copy)
                            else:
                                nc.scalar.activation(
                                    out=z_t[g][:, cc:cc + CH],
                                    in_=pps[(ob, c2)][:], func=AF.Copy)

            in_proj_part([0, 1, 2, 3], 'a')

            # ---- conv + silu -> xa; Wx + AllReduce per direction ----
            xa = {}
            for d in range(2):
                for g in range(NG):
                    base = 0 if d == 0 else PADL
                    acc = P.tile([128, L], BF16, tag='cva', bufs=2,
                                 name=f'cv0_{d}{g}')
                    nc.vector.tensor_scalar(
                        out=acc[:], in0=x_pad[g][:, base:base + L],
                        scalar1=convw_g[g][:, 4 * d:4 * d + 1], scalar2=None,
                        op0=ALU.mult)
                    for k in range(1, KC):
                        acc2 = P.tile([128, L], BF16, tag='cva', bufs=2,
                                      name=f'cvk{k}_{d}{g}')
                        nc.vector.scalar_tensor_tensor(
                            out=acc2[:], in0=x_pad[g][:, base + k:base + k + L],
                            scalar=convw_g[g][:, 4 * d + k:4 * d + k + 1],
                            in1=acc[:], op0=ALU.mult, op1=ALU.add)
                        acc = acc2
                    xat = P.tile([128, L], BF16, tag='xa', bufs=4,
                                 name=f'xa{d}{g}')
                    nc.scalar.activation(out=xat[:], in_=acc[:], func=AF.Silu,
                                         bias=convb_g[g][:, d:d + 1])
                    xa[(d, g)] = xat
                xdbl = P.tile([96, L], BF16, tag='xdbl', bufs=2,
                              name=f'xdbl{d}')
                for c in range(TC4):
                    pw = PS.tile([128, CH], F32, tag='bank', bufs=8,
                                 name=f'pw{d}_{c}')
                    for g in range(NG):
                        nc.tensor.matmul(
                            out=pw[0:96, :],
                            lhsT=wx_g[g][:, 96 * d:96 * (d + 1)],
                            rhs=xa[(d, g)][:, c * CH:(c + 1) * CH],
                            start=(g == 0), stop=(g == NG - 1))
                    nc.scalar.activation(out=xdbl[:, c * CH:(c + 1) * CH],
                                         in_=pw[0:96, :], func=AF.Copy)
                nc.sync.dma_start(out=t[f'ar_in{d}'][:], in_=xdbl[:])
                nc.gpsimd.collective_compute(
                    'AllReduce', ALU.add,
                    replica_groups=[list(range(NCORES))],
                    ins=[t[f'ar_in{d}'][:]], outs=[t[f'ar_out{d}'][:]])

            # ---- dtraw + dt/dtx for all (g, d) ----
            dtraw = []
            for d in range(2):
                dr = P.tile([64, L], BF16, tag='dtraw', bufs=2, name=f'dtraw{d}')
                nc.sync.dma_start(out=dr[:], in_=t[f'ar_out{d}'][0:64, :])
                dtraw.append(dr)

            # ---- dt-proj for all (g, d) ----
            dt_t = {}
            dtx_t = {}
            for d in range(2):
                for g in range(NG):
                    et = P.tile([128, L], BF16, tag='et', bufs=1,
                                name=f'et{g}{d}')
                    for c in range(TC4):
                        pd = PS.tile([128, CH], F32, tag='bank', bufs=8,
                                     name=f'pd{g}{d}{c}')
                        nc.tensor.matmul(
                            out=pd[:],
                            lhsT=wdt_t[:, d * DLOC + g * 128:
                                       d * DLOC + (g + 1) * 128],
                            rhs=dtraw[d][:, c * CH:(c + 1) * CH],
                            start=True, stop=True)
                        nc.scalar.activation(out=et[:, c * CH:(c + 1) * CH],
                                             in_=pd[:], func=AF.Exp,
                                             bias=bdt_g[g][:, d:d + 1])
                    dt = P.tile([128, L], BF16, tag='dt', bufs=4,
                                name=f'dt{g}{d}')
                    nc.scalar.activation(out=dt[:], in_=et[:], func=AF.Ln,
                                         bias=1.0)
                    dt_t[(g, d)] = dt
                    dtx = P.tile([128, L], BF16, tag='dtx', bufs=4,
                                 name=f'dtx{g}{d}')
                    nc.vector.tensor_tensor(out=dtx[:], in0=dt[:],
                                            in1=xa[(d, g)][:], op=ALU.mult)
                    dtx_t[(g, d)] = dtx

            # ---- main scan loop: d -> n -> g; py per g in PSUM ----
            py = [[PS.tile([128, CH], F32, tag='bank', bufs=8,
                           name=f'py{g}_{c}') for c in range(TC4)]
                  for g in range(NG)]
            for d in range(2):
                for n in range(N):
                    bc2 = P.tile([128, 2, L], BF16, tag='bc2', bufs=3,
                                 name=f'bc2{d}{n}')
                    src = t[f'ar_out{d}'][64 + n::16, :][0:2, :]
                    nc.sync.dma_start(
                        out=bc2[:],
                        in_=src.unsqueeze(0).to_broadcast([128, 2, L]))
                    bbc = bc2[:, 0, :]
                    cbc = bc2[:, 1, :]
                    for g in range(NG):
                        dA = P.tile([128, L], F32, tag='dA', bufs=2,
                                    name=f'dA{d}{n}{g}')
                        nc.scalar.activation(
                            out=dA[:], in_=dt_t[(g, d)][:], func=AF.Exp,
                            scale=asc_g[g][:, d * N + n:d * N + n + 1])
                        dBx = P.tile([128, L], BF16, tag='dBx', bufs=3,
                                     name=f'dBx{d}{n}{g}')
                        nc.vector.tensor_tensor(
                            out=dBx[:], in0=dtx_t[(g, d)][:], in1=bbc[:],
                            op=ALU.mult)
                        h_t = P.tile([128, L], BF16, tag='h', bufs=3,
                                     name=f'h{d}{n}{g}')
                        if d == 0:
                            nc.vector.tensor_tensor_scan(
                                out=h_t[:], data0=dA[:], data1=dBx[:],
                                initial=0.0, op0=ALU.mult, op1=ALU.add)
                        else:
                            nc.vector.tensor_tensor_scan(
                                out=h_t[:, ::-1], data0=dA[:, ::-1],
                                data1=dBx[:, ::-1],
                                initial=0.0, op0=ALU.mult, op1=ALU.add)
                        hc = P.tile([128, L], BF16, tag='hc', bufs=3,
                                    name=f'hc{d}{n}{g}')
                        nc.vector.tensor_tensor(
                            out=hc[:], in0=h_t[:], in1=cbc[:], op=ALU.mult)
                        for c in range(TC4):
                            nc.tensor.matmul(
                                out=py[g][c][:], lhsT=ident[:],
                                rhs=hc[:, c * CH:(c + 1) * CH],
                                start=(d == 0 and n == 0), stop=False)

            # ---- y tail: D-residual via PE, then gate ----
            for g in range(NG):
                for d in range(2):
                    for c in range(TC4):
                        nc.tensor.matmul(
                            out=py[g][c][:], lhsT=diag_d[(d, g)][:],
                            rhs=xa[(d, g)][:, c * CH:(c + 1) * CH],
                            start=False, stop=(d == 1))
            yg_t = {}
            for g in range(NG):
                sg = P.tile([128, L], BF16, tag='sg', bufs=1, name=f'sg{g}')
                nc.scalar.activation(out=sg[:], in_=z_t[g][:], func=AF.Silu)
                yg = P.tile([128, L], BF16, tag='yg', bufs=2, name=f'yg{g}')
                for c in range(TC4):
                    nc.vector.tensor_tensor(
                        out=yg[:, c * CH:(c + 1) * CH], in0=py[g][c][:],
                        in1=sg[:, c * CH:(c + 1) * CH], op=ALU.mult)
                yg_t[g] = yg

            # ---- out_proj: two mb-halves, each followed by its own RS ----
            for hf in range(2):
                for mb in range(hf * 4, hf * 4 + 4):
                    wos = []
                    for g in range(NG):
                        wo = P.tile([128, 128], BF16, tag='wstream', bufs=8,
                                    name=f'wo{mb}_{g}')
                        nc.gpsimd.dma_start(
                            out=wo[:],
                            in_=t['w_out_T'][g * 128:(g + 1) * 128,
                                             mb * 128:(mb + 1) * 128])
                        wos.append(wo)
                    for c in range(TC4):
                        po = PS.tile([128, CH], F32, tag='bank', bufs=8,
                                     name=f'po{mb}{c}')
                        for g in range(NG):
                            nc.tensor.matmul(
                                out=po[:], lhsT=wos[g][:],
                                rhs=yg_t[g][:, c * CH:(c + 1) * CH],
                                start=(g == 0), stop=(g == NG - 1))
                        ost = P.tile([128, CH], BF16, tag='ost', bufs=4,
                                     name=f'os{mb}{c}')
                        nc.scalar.activation(out=ost[:], in_=po[:],
                                             func=AF.Copy)
                        nc.sync.dma_start(
                            out=t[f'rs_in{hf}'][(mb - hf * 4) * 128:
                                                (mb - hf * 4 + 1) * 128,
                                                c * CH:(c + 1) * CH],
                            in_=ost[:])
                nc.gpsimd.collective_compute(
                    'ReduceScatter', ALU.add,
                    replica_groups=[list(range(NCORES))],
                    ins=[t[f'rs_in{hf}'][:]], outs=[t[f'rs_out{hf}'][:]])
                nc.sync.dma_start(out=t[f'out_bT{hf}'][:],
                                  in_=t[f'rs_out{hf}'][:])


def _build():
    nc = bacc.Bacc(None, target_bir_lowering=False)

    def inp(name, shape, dt=F32):
        return nc.declare_dram_parameter(name, shape, dt, isOutput=False)

    t = {
        'h_T': inp('h_T', [DM, L], BF16),
        'w_in_T': inp('w_in_T', [DM, 2 * DLOC], BF16),
        'wx_T': inp('wx_T', [DLOC, 2 * 96], BF16),
        'wdt_T': inp('wdt_T', [R, 2 * DLOC], BF16),
        'w_out_T': inp('w_out_T', [DLOC, DM], BF16),
        'bdt': inp('bdt', [DLOC, 2]),
        'conv_w': inp('conv_w', [DLOC, 2 * KC]),
        'conv_b': inp('conv_b', [DLOC, 2]),
        'd_both': inp('d_both', [DLOC, 2]),
        'a_sc': inp('a_sc', [DLOC, 2 * N]),
        'ident': inp('ident', [128, 128], BF16),
        'diag_d': inp('diag_d', [512, 128], BF16),
        'out_bT0': nc.declare_dram_parameter('out_bT0', [64, L], BF16,
                                             isOutput=True),
        'out_bT1': nc.declare_dram_parameter('out_bT1', [64, L], BF16,
                                             isOutput=True),
        'ar_in0': nc.dram_tensor('ar_in0', [96, L], BF16),
        'ar_in1': nc.dram_tensor('ar_in1', [96, L], BF16),
        'ar_out0': nc.dram_tensor('ar_out0', [96, L], BF16,
                                  addr_space='Shared'),
        'ar_out1': nc.dram_tensor('ar_out1', [96, L], BF16,
                                  addr_space='Shared'),
        'rs_in0': nc.dram_tensor('rs_in0', [DM // 2, L], BF16),
        'rs_in1': nc.dram_tensor('rs_in1', [DM // 2, L], BF16),
        'rs_out0': nc.dram_tensor('rs_out0', [64, L], BF16),
        'rs_out1': nc.dram_tensor('rs_out1', [64, L], BF16),
    }
    _emit(nc, t)
    nc.compile()
    return nc


def prepare_in_maps(inputs):
    import ml_dtypes
    f32 = np.float32
    bf16 = ml_dtypes.bfloat16
    h = np.asarray(inputs['hidden_states'], f32)[0]        # [L, DM]
    h_T = np.ascontiguousarray(h.T).astype(bf16)
    W_in = np.asarray(inputs['W_in'], f32)
    W_out = np.asarray(inputs['W_out'], f32)
    ident = np.eye(128, dtype=f32).astype(bf16)

    maps = []
    for c in range(NCORES):
        sl = slice(c * DLOC, (c + 1) * DLOC)
        cw_f = np.asarray(inputs['conv_w_fwd'], f32)[sl]          # natural taps
        cw_r = np.asarray(inputs['conv_w_rev'], f32)[sl][:, ::-1]  # flipped
        a_sc = np.concatenate(
            [-np.exp(np.asarray(inputs['A_log_fwd'], f32)[sl]),
             -np.exp(np.asarray(inputs['A_log_rev'], f32)[sl])], axis=1)
        m = {
            'h_T': h_T,
            'w_in_T': np.ascontiguousarray(np.concatenate(
                [W_in[sl].T, W_in[DI + c * DLOC:DI + (c + 1) * DLOC].T],
                axis=1)).astype(bf16),
            'wx_T': np.ascontiguousarray(np.concatenate(
                [np.asarray(inputs['Wx_fwd'], f32)[:, sl].T,
                 np.asarray(inputs['Wx_rev'], f32)[:, sl].T],
                axis=1)).astype(bf16),
            'wdt_T': np.ascontiguousarray(np.concatenate(
                [np.asarray(inputs['Wdt_fwd'], f32)[sl].T,
                 np.asarray(inputs['Wdt_rev'], f32)[sl].T],
                axis=1)).astype(bf16),
            'w_out_T': np.ascontiguousarray(W_out[:, sl].T).astype(bf16),
            'bdt': np.ascontiguousarray(np.stack(
                [np.asarray(inputs['bdt_fwd'], f32)[sl],
                 np.asarray(inputs['bdt_rev'], f32)[sl]], axis=1)),
            'conv_w': np.ascontiguousarray(
                np.concatenate([cw_f, cw_r], axis=1)),
            'conv_b': np.ascontiguousarray(np.stack(
                [np.asarray(inputs['conv_b_fwd'], f32)[sl],
                 np.asarray(inputs['conv_b_rev'], f32)[sl]], axis=1)),
            'd_both': np.ascontiguousarray(np.stack(
                [np.asarray(inputs['D_fwd'], f32)[sl],
                 np.asarray(inputs['D_rev'], f32)[sl]], axis=1)),
            'a_sc': np.ascontiguousarray(a_sc),
            'ident': ident,
            'diag_d': np.ascontiguousarray(np.concatenate(
                [np.diag(np.asarray(inputs[f'D_{dd}'], f32)
                         [c * DLOC + gg * 128:c * DLOC + (gg + 1) * 128])
                 for dd in ('fwd', 'rev') for gg in range(NG)],
                axis=0)).astype(bf16),
        }
        maps.append(m)
    return maps


def get_nc():
    if 'nc' not in _CACHE:
        _CACHE['nc'] = _build()
    return _CACHE['nc']


def run(inputs, **kw):
    nc = get_nc()
    maps = prepare_in_maps(inputs)
    res = run_bass_kernel_spmd(nc, maps, list(range(NCORES)), **kw)
    out_T = np.empty((DM, L), np.float32)
    for c in range(NCORES):
        out_T[c * 64:(c + 1) * 64] = \
            np.asarray(res.results[c]['out_bT0']).astype(np.float32)
        out_T[DM // 2 + c * 64:DM // 2 + (c + 1) * 64] = \
            np.asarray(res.results[c]['out_bT1']).astype(np.float32)
    out = np.ascontiguousarray(out_T.T)[None]             # [1, L, DM]
    return out.astype(np.float32), res


def kernel(**inputs):
    out, _ = run(inputs)
    return out
